# revision 19
# baseline (speedup 1.0000x reference)
"""Channel-attention module kernel for 8 Trainium2 NeuronCores.

reference semantics (B=2, C=128, N=D*H*W=147456):
    q = x.reshape(B, C, N)
    energy = q @ q^T                  # [B, C, C]
    attn = softmax(rowmax(energy) - energy, axis=-1)
          = softmax(-energy, axis=-1)             (rowmax shift is a no-op)
    out = attn @ q
    return x + gamma * out

Sharding: sequence-parallel over N. Core r owns columns
[r*N/8, (r+1)*N/8) of q for both batches. Each core computes a partial
energy (contraction over its local n), a per-batch AllReduce sums the
tiny [C, C] energy across the 8 cores, each core computes the softmax
redundantly and applies the attention to its local columns.

Precision scheme (hw-validated):
  - Host splits x = hi + lo into two bf16 planes (lo = bf16(x - hi)).
  - energy = hi@hi^T + S + S^T where S = hi@lo^T, accumulated in fp32
    PSUM from bf16 matmuls (bf16 products are exact in fp32). The
    dropped lo@lo^T term is ~5e-4 off-diagonal (vs the smallest softmax
    argmin gap of 0.03 on these inputs) and ~0.2 on the diagonal, which
    softmax ignores (the +N diagonal is always the most-negative logit).
  - The transposed [n, C] tiles that the energy contraction needs come
    straight from the DMA xbar (2-byte transpose, one blocked-3D
    transpose instruction per [128, 2048] chunk) — no TensorE
    transposes, no PSUM-evacuation copies.
  - Phase 2 (attn apply) runs in bf16 with the residual folded into the
    attention matrix (attn_s = gamma/Z * P + I; P's diagonal is exactly
    0), outputs stored as fp16 (~1e-4 extra error, halves store bytes).

Pipelining: energy(b0) -> AR(b0) overlaps energy(b1); AR(b1) overlaps
phase2(b0). A dummy AllReduce at kernel start absorbs the ~45us ncfw
cold-start so the real ARs run at the ~10us floor.
"""

import sys

sys.path.insert(0, "/opt/trn_rl_repo")

import numpy as np

B, C = 2, 128
D, H, W = 16, 96, 96
N = D * H * W  # 147456
NCORES = 8
NLOC = N // NCORES  # 18432
CHUNK = 2048
NCHUNK = NLOC // CHUNK  # 9
OTILE = 512

_compiled = {}


def _log(msg):
    import time as _t
    print(f"[kernel {_t.strftime('%H:%M:%S')}] {msg}", flush=True)


def _build():
    import concourse.bacc as bacc
    import concourse.tile as tile
    import concourse.mybir as mybir

    _log("build start")

    f32 = mybir.dt.float32
    f16 = mybir.dt.float16
    bf16 = mybir.dt.bfloat16
    nc = bacc.Bacc("TRN2", target_bir_lowering=False, debug=False,
                   num_devices=NCORES)

    xh_d = nc.dram_tensor("xh", [B, C, NLOC], bf16, kind="ExternalInput").ap()
    xl_d = nc.dram_tensor("xl", [B, C, NLOC], bf16, kind="ExternalInput").ap()
    g_d = nc.dram_tensor("gamma_col", [C, 1], f32, kind="ExternalInput").ap()
    id_d = nc.dram_tensor("ident", [C, C], f32, kind="ExternalInput").ap()
    o_d = nc.dram_tensor("out", [B, C, NLOC], f16, kind="ExternalOutput").ap()

    NBLK = CHUNK // C  # 16 transposed blocks per chunk

    with tile.TileContext(nc) as tc:
        with (
            tc.tile_pool(name="qhring", bufs=4) as qhp,
            tc.tile_pool(name="qlring", bufs=4) as qlp,
            tc.tile_pool(name="xnat", bufs=B * NCHUNK) as xnp_,
            tc.tile_pool(name="epsA", bufs=2, space="PSUM") as epsA,
            tc.tile_pool(name="epsS", bufs=2, space="PSUM") as epsS,
            tc.tile_pool(name="tps", bufs=2, space="PSUM") as tps,
            tc.tile_pool(name="ops", bufs=2, space="PSUM") as ops,
            tc.tile_pool(name="misc", bufs=1) as mp,
            tc.tile_pool(name="ost", bufs=3) as ostp,
            tc.tile_pool(name="dram", bufs=1, space="DRAM") as dramp,
        ):
            ident = mp.tile([C, C], f32, name="ident_sb")
            nc.sync.dma_start(ident[:], id_d[:])
            gcol = mp.tile([C, 1], f32, name="gcol")
            nc.sync.dma_start(gcol[:], g_d[:])

            # Warm-up collective: the FIRST collective on this runtime pays
            # a ~45us ncfw cold-start (hw-measured); later ones hit the
            # ~10us floor.
            w_in = dramp.tile([C, 1], f32, name="w_in")
            w_out = dramp.tile([C, 1], f32, name="w_out", addr_space="Shared")
            nc.gpsimd.dma_start(w_in[:], gcol[:])
            nc.gpsimd.collective_compute(
                "AllReduce", mybir.AluOpType.add,
                replica_groups=[list(range(NCORES))],
                ins=[w_in.opt()], outs=[w_out.opt()],
            )

            xnat = [[xnp_.tile([C, CHUNK], bf16, name=f"xn_{b}_{k}", tag="xn")
                     for k in range(NCHUNK)] for b in range(B)]

            # ---- phase 1: blocked-transposed loads + 2 bf16 matmuls/tile ----
            ntile = NCHUNK * NBLK  # 144 n-tiles of 128 per batch
            E_sb = []
            for b in range(B):
                eA = epsA.tile([C, C], f32, name=f"eA{b}", tag="eA")
                eS = epsS.tile([C, C], f32, name=f"eS{b}", tag="eS")
                t = 0
                for k in range(NCHUNK):
                    sl = slice(k * CHUNK, (k + 1) * CHUNK)
                    qh = qhp.tile([C, CHUNK], bf16, name=f"qh_{b}_{k}",
                                  tag="qh")
                    qh3 = qh.rearrange("p (blk c) -> p blk c", blk=NBLK)
                    ql = qlp.tile([C, CHUNK], bf16, name=f"ql_{b}_{k}",
                                  tag="ql")
                    ql3 = ql.rearrange("p (blk c) -> p blk c", blk=NBLK)
                    if b == 0 and k == 0:
                        # split the first transposed loads so PE starts early
                        for s in range(2):
                            half = slice(s * (CHUNK // 2),
                                         (s + 1) * (CHUNK // 2))
                            nc.sync.dma_start(
                                qh3[:, s * (NBLK // 2):(s + 1) * (NBLK // 2),
                                    :],
                                xh_d[0, :, half], transpose=True)
                            nc.sync.dma_start(
                                ql3[:, s * (NBLK // 2):(s + 1) * (NBLK // 2),
                                    :],
                                xl_d[0, :, half], transpose=True)
                    else:
                        nc.sync.dma_start(qh3[:], xh_d[b, :, sl],
                                          transpose=True)
                        nc.sync.dma_start(ql3[:], xl_d[b, :, sl],
                                          transpose=True)
                    for j in range(NBLK):
                        Ht = qh[:, j * C:(j + 1) * C]
                        Lt = ql[:, j * C:(j + 1) * C]
                        nc.tensor.matmul(eA[:], Ht, Ht,
                                         start=(t == 0), stop=(t == ntile - 1))
                        nc.tensor.matmul(eS[:], Ht, Lt,
                                         start=(t == 0), stop=(t == ntile - 1))
                        t += 1

                # E = A + S + S^T  (lo@lo^T dropped; see module docstring)
                S_sb = mp.tile([C, C], f32, name=f"S_sb{b}")
                nc.vector.tensor_copy(S_sb[:], eS[:])
                e_cat = mp.tile([C, C], f32, name=f"e_cat{b}")
                nc.vector.tensor_tensor(e_cat[:], S_sb[:], eA[:],
                                        op=mybir.AluOpType.add)
                tpS = tps.tile([C, C], f32, name=f"tpS{b}", tag="tp")
                nc.tensor.transpose(tpS[:], S_sb[:], ident[:])
                nc.vector.tensor_add(e_cat[:], e_cat[:], tpS[:])

                ar_in = dramp.tile([C, C], f32, name=f"ar_in{b}")
                ar_out = dramp.tile([C, C], f32, name=f"ar_out{b}",
                                    addr_space="Shared")
                nc.gpsimd.dma_start(ar_in[:], e_cat[:])
                nc.gpsimd.collective_compute(
                    "AllReduce", mybir.AluOpType.add,
                    replica_groups=[list(range(NCORES))],
                    ins=[ar_in.opt()], outs=[ar_out.opt()],
                )
                e_red = mp.tile([C, C], f32, name=f"e_red{b}")
                nc.gpsimd.dma_start(e_red[:], ar_out[:])
                E_sb.append(e_red)

                # natural-layout bf16 x for phase 2; loads overlap the next
                # stretch of compute
                for k in range(NCHUNK):
                    nc.sync.dma_start(xnat[b][k][:],
                                      xh_d[b, :, k * CHUNK:(k + 1) * CHUNK])

            # ---- phase 2: softmax + apply, per batch ----
            def emit_softmax(b):
                E_b = E_sb[b][:]
                mcol = mp.tile([C, 1], f32, name=f"mcol{b}")
                nc.vector.tensor_reduce(mcol[:], E_b, axis=mybir.AxisListType.X,
                                        op=mybir.AluOpType.min)
                P_b = mp.tile([C, C], f32, name=f"P{b}")
                zcol = mp.tile([C, 1], f32, name=f"zcol{b}")
                # P = exp(min_row - E), zcol = rowsum(P); exponents <= 0.
                # P's diagonal is exp(min - ~+147000) == 0 exactly.
                nc.scalar.activation(P_b[:], E_b,
                                     mybir.ActivationFunctionType.Exp,
                                     bias=mcol[:], scale=-1.0,
                                     accum_out=zcol[:])
                rz = mp.tile([C, 1], f32, name=f"rz{b}")
                nc.vector.reciprocal(rz[:], zcol[:])
                scol = mp.tile([C, 1], f32, name=f"scol{b}")
                nc.vector.tensor_tensor(scol[:], rz[:], gcol[:],
                                        op=mybir.AluOpType.mult)
                # attn_s = (gamma/Z) * P + I -> matmul computes x + gamma*attn@q
                nc.vector.tensor_scalar_mul(P_b[:], P_b[:], scol[:])
                nc.vector.tensor_add(P_b[:], P_b[:], ident[:])
                tp2 = tps.tile([C, C], f32, name=f"tpP{b}", tag="tp")
                nc.tensor.transpose(tp2[:], P_b[:], ident[:])
                attnT = mp.tile([C, C], bf16, name=f"attnT{b}")
                nc.vector.tensor_copy(attnT[:], tp2[:])  # fp32 psum -> bf16
                return attnT

            def emit_apply_chunk(b, attnT, k):
                ost = ostp.tile([C, CHUNK], f16, name=f"ost_{b}_{k}",
                                tag="ost")
                for j in range(CHUNK // OTILE):
                    op = ops.tile([C, OTILE], f32, name=f"op_{b}_{k}_{j}",
                                  tag="op")
                    nc.tensor.matmul(
                        op[:], attnT[:],
                        xnat[b][k][:, j * OTILE:(j + 1) * OTILE],
                        start=True, stop=True)
                    nc.vector.tensor_copy(
                        ost[:, j * OTILE:(j + 1) * OTILE], op[:])
                nc.sync.dma_start(o_d[b, :, k * CHUNK:(k + 1) * CHUNK],
                                  ost[:])

            attnT0 = emit_softmax(0)
            attnT1 = None
            for k in range(NCHUNK):
                emit_apply_chunk(0, attnT0, k)
                if k == 5:
                    # AR(b1) has completed by now; emitting softmax(b1) here
                    # keeps its DVE chain from queueing behind all of
                    # p2b0's evac copies.
                    attnT1 = emit_softmax(1)
            for k in range(NCHUNK):
                emit_apply_chunk(1, attnT1, k)

    _log("tile context done; bacc compile start")
    nc.compile()
    _log("bacc compile done")
    return nc


def _get_nc():
    if "nc" not in _compiled:
        _compiled["nc"] = _build()
    return _compiled["nc"]


def kernel(x, gamma, _trace=False, _tmpdir=None):
    import ml_dtypes
    from concourse import bass_utils

    x = np.ascontiguousarray(np.asarray(x), dtype=np.float32)
    gamma = np.asarray(gamma, dtype=np.float32)
    q = x.reshape(B, C, N)
    q_hi = q.astype(ml_dtypes.bfloat16)
    q_lo = (q - q_hi.astype(np.float32)).astype(ml_dtypes.bfloat16)
    gcol = np.full((C, 1), gamma[0], dtype=np.float32)
    ident = np.eye(C, dtype=np.float32)

    in_maps = []
    for r in range(NCORES):
        sl = slice(r * NLOC, (r + 1) * NLOC)
        in_maps.append({
            "xh": np.ascontiguousarray(q_hi[:, :, sl]),
            "xl": np.ascontiguousarray(q_lo[:, :, sl]),
            "gamma_col": gcol,
            "ident": ident,
        })

    nc = _get_nc()
    _log("launching run_bass_kernel_spmd")
    res = bass_utils.run_bass_kernel_spmd(
        nc, in_maps, core_ids=list(range(NCORES)), trace=_trace,
        tmpdir=_tmpdir)
    outs = [res.results[r]["out"] for r in range(NCORES)]
    full = np.concatenate(outs, axis=2).astype(np.float32)
    full = full.reshape(B, C, D, H, W)
    if _trace:
        return full, res
    return full


# revision 22
# speedup vs baseline: 1.3830x; 1.3830x over previous
"""Channel-attention module kernel for 8 Trainium2 NeuronCores.

reference semantics (B=2, C=128, N=D*H*W=147456):
    q = x.reshape(B, C, N)
    energy = q @ q^T                  # [B, C, C]
    attn = softmax(rowmax(energy) - energy, axis=-1)
          = softmax(-energy, axis=-1)             (rowmax shift is a no-op)
    out = attn @ q
    return x + gamma * out

Sharding: sequence-parallel over N. Core r owns columns
[r*N/8, (r+1)*N/8) of q for both batches. Each core computes a partial
energy (contraction over its local n), a per-batch AllReduce sums the
tiny [C, C] energy across the 8 cores, each core then computes the
softmax redundantly and applies the attention to its local columns.

Pipelining: energy(b0) -> AR(b0) overlaps energy(b1); AR(b1) overlaps
phase2(b0).

Precision split:
  - energy contraction: true fp32 (softmax argmin gaps as small as 0.03
    on these inputs; one argmin flip alone is ~5% global rel err).
  - phase 2 (attn apply): bf16. The residual is folded into the
    attention matrix (attn_s = gamma/Z * P + I; P's diagonal is exactly
    0 because the energy diagonal ~ +N dominates), so phase 2 is
    out = attn_s @ q with q rounded to bf16 — error is linear, ~0.4%,
    far inside the 2e-2 gate. This makes phase-2 matmuls 4x faster than
    fp32 and lets the fp32 x chunks be freed after phase 1: x lives in
    a small fp32 ring; a resident bf16 copy (cast on the idle ScalarE
    during phase 1) feeds phase 2.
"""

import sys

sys.path.insert(0, "/opt/trn_rl_repo")

import numpy as np

B, C = 2, 128
D, H, W = 16, 96, 96
N = D * H * W  # 147456
NCORES = 8
NLOC = N // NCORES  # 18432
CHUNK = 2048
NCHUNK = NLOC // CHUNK  # 9
OTILE = 512
PIPE = 2  # transposes emitted ahead of their matmul (keeps PE fed)

_compiled = {}


def _log(msg):
    import time as _t
    print(f"[kernel {_t.strftime('%H:%M:%S')}] {msg}", flush=True)


def _build():
    import concourse.bacc as bacc
    import concourse.tile as tile
    import concourse.mybir as mybir

    _log("build start")

    f32 = mybir.dt.float32
    bf16 = mybir.dt.bfloat16
    nc = bacc.Bacc("TRN2", target_bir_lowering=False, debug=False,
                   num_devices=NCORES)

    x_d = nc.dram_tensor("x", [B, C, NLOC], f32, kind="ExternalInput").ap()
    g_d = nc.dram_tensor("gamma_col", [C, 1], f32, kind="ExternalInput").ap()
    id_d = nc.dram_tensor("ident", [C, C], f32, kind="ExternalInput").ap()
    o_d = nc.dram_tensor("out", [B, C, NLOC], f32, kind="ExternalOutput").ap()

    with tile.TileContext(nc) as tc:
        with (
            tc.tile_pool(name="xring", bufs=7) as xp,
            tc.tile_pool(name="xb16", bufs=B * NCHUNK) as xbp,
            tc.tile_pool(name="qt", bufs=6) as qtp,
            tc.tile_pool(name="tps", bufs=3, space="PSUM") as tps,
            tc.tile_pool(name="eps", bufs=2, space="PSUM") as eps,
            tc.tile_pool(name="ops", bufs=3, space="PSUM") as ops,
            tc.tile_pool(name="misc", bufs=1) as mp,
            tc.tile_pool(name="ost", bufs=3) as ostp,
            tc.tile_pool(name="dram", bufs=1, space="DRAM") as dramp,
        ):
            ident = mp.tile([C, C], f32, name="ident_sb")
            nc.sync.dma_start(ident[:], id_d[:])
            gcol = mp.tile([C, 1], f32, name="gcol")
            nc.sync.dma_start(gcol[:], g_d[:])

            # Warm-up collective: the FIRST collective on this runtime pays
            # a ~45us ncfw cold-start (hw-measured); later ones hit the
            # ~10us floor. Fire a tiny dummy AllReduce immediately so the
            # real per-batch AllReduces run warm.
            w_in = dramp.tile([C, 1], f32, name="w_in")
            w_out = dramp.tile([C, 1], f32, name="w_out", addr_space="Shared")
            nc.gpsimd.dma_start(w_in[:], gcol[:])
            nc.gpsimd.collective_compute(
                "AllReduce", mybir.AluOpType.add,
                replica_groups=[list(range(NCORES))],
                ins=[w_in.opt()], outs=[w_out.opt()],
            )

            xb16 = [[xbp.tile([C, CHUNK], bf16, name=f"xb_{b}_{k}", tag="xb")
                     for k in range(NCHUNK)] for b in range(B)]

            # ---- phase 1 + per-batch AllReduce ----
            ntile_c = CHUNK // C  # 16 n-tiles of 128 per chunk
            ntile = NCHUNK * ntile_c  # 144 per batch
            E_sb = []
            for b in range(B):
                e_ps = eps.tile([C, C], f32, name=f"e_ps{b}", tag="e")
                pend = []
                mm = 0

                def flush(e_ps=e_ps):
                    nonlocal mm
                    qt = pend.pop(0)
                    nc.tensor.matmul(e_ps[:], qt[:], qt[:],
                                     start=(mm == 0), stop=(mm == ntile - 1))
                    mm += 1

                for k in range(NCHUNK):
                    xt = xp.tile([C, CHUNK], f32, name=f"x_{b}_{k}", tag="x")
                    src = x_d[b, :, k * CHUNK:(k + 1) * CHUNK]
                    # alternate the two HWDGE rings (sync / scalar) so
                    # consecutive loads don't queue head-of-line on one ring
                    dmae = nc.sync if k % 2 == 0 else nc.scalar
                    if b == 0 and k == 0:
                        # split the very first load so PE can start early
                        for s in range(2):
                            (nc.sync if s == 0 else nc.scalar).dma_start(
                                xt[:, s * 1024:(s + 1) * 1024],
                                x_d[0, :, s * 1024:(s + 1) * 1024])
                    else:
                        dmae.dma_start(xt[:], src)
                    for j in range(ntile_c):
                        t = k * ntile_c + j
                        tp = tps.tile([C, C], f32, name=f"tp_{b}_{t}",
                                      tag="tp")
                        nc.tensor.transpose(
                            tp[:], xt[:, j * C:(j + 1) * C], ident[:])
                        qt = qtp.tile([C, C], f32, name=f"qt_{b}_{t}",
                                      tag="qt")
                        nc.vector.tensor_copy(qt[:], tp[:])
                        pend.append(qt)
                        if len(pend) > PIPE:
                            flush()
                    # bf16 copy for phase 2 (ScalarE is idle in phase 1);
                    # after this the fp32 ring slot can be reused.
                    nc.scalar.copy(xb16[b][k][:], xt[:])
                while pend:
                    flush()
                e_cat = mp.tile([C, C], f32, name=f"e_cat{b}")
                nc.vector.tensor_copy(e_cat[:], e_ps[:])

                ar_in = dramp.tile([C, C], f32, name=f"ar_in{b}")
                ar_out = dramp.tile([C, C], f32, name=f"ar_out{b}",
                                    addr_space="Shared")
                # bounce DMAs on GPSIMD/SWDGE: the HWDGE (sync) ring is
                # strictly FIFO, so a collective-gated load there would
                # block all later chunk loads / output stores.
                nc.gpsimd.dma_start(ar_in[:], e_cat[:])
                nc.gpsimd.collective_compute(
                    "AllReduce", mybir.AluOpType.add,
                    replica_groups=[list(range(NCORES))],
                    ins=[ar_in.opt()], outs=[ar_out.opt()],
                )
                e_red = mp.tile([C, C], f32, name=f"e_red{b}")
                nc.gpsimd.dma_start(e_red[:], ar_out[:])
                E_sb.append(e_red)

            # ---- phase 2: softmax + apply, per batch ----
            def emit_softmax(b):
                E_b = E_sb[b][:]
                mcol = mp.tile([C, 1], f32, name=f"mcol{b}")
                nc.vector.tensor_reduce(mcol[:], E_b, axis=mybir.AxisListType.X,
                                        op=mybir.AluOpType.min)
                P_b = mp.tile([C, C], f32, name=f"P{b}")
                zcol = mp.tile([C, 1], f32, name=f"zcol{b}")
                # P = exp(min_row - E), zcol = rowsum(P); exponents <= 0.
                # P's diagonal is exp(min - ~+147000) == 0 exactly.
                nc.scalar.activation(P_b[:], E_b,
                                     mybir.ActivationFunctionType.Exp,
                                     bias=mcol[:], scale=-1.0,
                                     accum_out=zcol[:])
                rz = mp.tile([C, 1], f32, name=f"rz{b}")
                nc.vector.reciprocal(rz[:], zcol[:])
                scol = mp.tile([C, 1], f32, name=f"scol{b}")
                nc.vector.tensor_tensor(scol[:], rz[:], gcol[:],
                                        op=mybir.AluOpType.mult)
                # attn_s = (gamma/Z) * P + I  -> matmul computes x + gamma*attn@q
                nc.vector.tensor_scalar_mul(P_b[:], P_b[:], scol[:])
                nc.vector.tensor_add(P_b[:], P_b[:], ident[:])
                tp2 = tps.tile([C, C], f32, name=f"tpP{b}", tag="tp")
                nc.tensor.transpose(tp2[:], P_b[:], ident[:])
                attnT = mp.tile([C, C], bf16, name=f"attnT{b}")
                nc.vector.tensor_copy(attnT[:], tp2[:])  # fp32 psum -> bf16
                return attnT

            def emit_apply_chunk(b, attnT, k):
                ost = ostp.tile([C, CHUNK], f32, name=f"ost_{b}_{k}",
                                tag="ost")
                for j in range(CHUNK // OTILE):
                    op = ops.tile([C, OTILE], f32, name=f"op_{b}_{k}_{j}",
                                  tag="op")
                    nc.tensor.matmul(
                        op[:], attnT[:],
                        xb16[b][k][:, j * OTILE:(j + 1) * OTILE],
                        start=True, stop=True)
                    nc.vector.tensor_copy(
                        ost[:, j * OTILE:(j + 1) * OTILE], op[:])
                (nc.sync if k % 2 == 0 else nc.scalar).dma_start(
                    o_d[b, :, k * CHUNK:(k + 1) * CHUNK], ost[:])

            attnT0 = emit_softmax(0)
            attnT1 = None
            for k in range(NCHUNK):
                emit_apply_chunk(0, attnT0, k)
                if k == 5:
                    # AR(b1) has completed by now; emitting softmax(b1) here
                    # keeps its DVE chain from queueing behind all of
                    # p2b0's evac copies.
                    attnT1 = emit_softmax(1)
            for k in range(NCHUNK):
                emit_apply_chunk(1, attnT1, k)

    _log("tile context done; bacc compile start")
    nc.compile()
    _log("bacc compile done")
    return nc


def _get_nc():
    if "nc" not in _compiled:
        _compiled["nc"] = _build()
    return _compiled["nc"]


def kernel(x, gamma, _trace=False, _tmpdir=None):
    from concourse import bass_utils

    x = np.ascontiguousarray(np.asarray(x), dtype=np.float32)
    gamma = np.asarray(gamma, dtype=np.float32)
    q = x.reshape(B, C, N)
    gcol = np.full((C, 1), gamma[0], dtype=np.float32)
    ident = np.eye(C, dtype=np.float32)

    in_maps = []
    for r in range(NCORES):
        in_maps.append({
            "x": np.ascontiguousarray(q[:, :, r * NLOC:(r + 1) * NLOC]),
            "gamma_col": gcol,
            "ident": ident,
        })

    nc = _get_nc()
    _log("launching run_bass_kernel_spmd")
    res = bass_utils.run_bass_kernel_spmd(
        nc, in_maps, core_ids=list(range(NCORES)), trace=_trace,
        tmpdir=_tmpdir)
    outs = [res.results[r]["out"] for r in range(NCORES)]
    full = np.concatenate(outs, axis=2).reshape(B, C, D, H, W)
    if _trace:
        return full.astype(np.float32, copy=False), res
    return full.astype(np.float32, copy=False)


# revision 24
# speedup vs baseline: 1.4634x; 1.0581x over previous
"""Channel-attention module kernel for 8 Trainium2 NeuronCores.

reference semantics (B=2, C=128, N=D*H*W=147456):
    q = x.reshape(B, C, N)
    energy = q @ q^T                  # [B, C, C]
    attn = softmax(rowmax(energy) - energy, axis=-1)
          = softmax(-energy, axis=-1)             (rowmax shift is a no-op)
    out = attn @ q
    return x + gamma * out

Sharding: sequence-parallel over N. Core r owns columns
[r*N/8, (r+1)*N/8) of q for both batches. Each core computes a partial
energy (contraction over its local n), a per-batch AllReduce sums the
tiny [C, C] energy across the 8 cores, each core then computes the
softmax redundantly and applies the attention to its local columns.

Pipelining: energy(b0) -> AR(b0) overlaps energy(b1); AR(b1) overlaps
phase2(b0).

Precision split:
  - energy contraction: true fp32 (softmax argmin gaps as small as 0.03
    on these inputs; one argmin flip alone is ~5% global rel err).
  - phase 2 (attn apply): bf16. The residual is folded into the
    attention matrix (attn_s = gamma/Z * P + I; P's diagonal is exactly
    0 because the energy diagonal ~ +N dominates), so phase 2 is
    out = attn_s @ q with q rounded to bf16 — error is linear, ~0.4%,
    far inside the 2e-2 gate. This makes phase-2 matmuls 4x faster than
    fp32 and lets the fp32 x chunks be freed after phase 1: x lives in
    a small fp32 ring; a resident bf16 copy (cast on the idle ScalarE
    during phase 1) feeds phase 2.
"""

import sys

sys.path.insert(0, "/opt/trn_rl_repo")

import numpy as np

B, C = 2, 128
D, H, W = 16, 96, 96
N = D * H * W  # 147456
NCORES = 8
NLOC = N // NCORES  # 18432
CHUNK = 2048
NCHUNK = NLOC // CHUNK  # 9
OTILE = 512
PIPE = 2  # transposes emitted ahead of their matmul (keeps PE fed)

_compiled = {}


def _log(msg):
    import time as _t
    print(f"[kernel {_t.strftime('%H:%M:%S')}] {msg}", flush=True)


def _build():
    import concourse.bacc as bacc
    import concourse.tile as tile
    import concourse.mybir as mybir

    _log("build start")

    f32 = mybir.dt.float32
    bf16 = mybir.dt.bfloat16
    nc = bacc.Bacc("TRN2", target_bir_lowering=False, debug=False,
                   num_devices=NCORES)

    x_d = nc.dram_tensor("x", [B, C, NLOC], f32, kind="ExternalInput").ap()
    g_d = nc.dram_tensor("gamma_col", [C, 1], f32, kind="ExternalInput").ap()
    id_d = nc.dram_tensor("ident", [C, C], f32, kind="ExternalInput").ap()
    o_d = nc.dram_tensor("out", [B, C, NLOC], f32, kind="ExternalOutput").ap()

    with tile.TileContext(nc) as tc:
        with (
            tc.tile_pool(name="xring", bufs=7) as xp,
            tc.tile_pool(name="xb16", bufs=B * NCHUNK) as xbp,
            tc.tile_pool(name="qt", bufs=6) as qtp,
            tc.tile_pool(name="tps", bufs=3, space="PSUM") as tps,
            tc.tile_pool(name="eps", bufs=2, space="PSUM") as eps,
            tc.tile_pool(name="ops", bufs=3, space="PSUM") as ops,
            tc.tile_pool(name="misc", bufs=1) as mp,
            tc.tile_pool(name="ost", bufs=3) as ostp,
            tc.tile_pool(name="dram", bufs=1, space="DRAM") as dramp,
        ):
            ident = mp.tile([C, C], f32, name="ident_sb")
            nc.sync.dma_start(ident[:], id_d[:])
            gcol = mp.tile([C, 1], f32, name="gcol")
            nc.sync.dma_start(gcol[:], g_d[:])

            # Warm-up collective: the FIRST collective on this runtime pays
            # a ~45us ncfw cold-start (hw-measured); later ones hit the
            # ~10us floor. Fire a tiny dummy AllReduce immediately so the
            # real per-batch AllReduces run warm.
            w_in = dramp.tile([C, 1], f32, name="w_in")
            w_out = dramp.tile([C, 1], f32, name="w_out", addr_space="Shared")
            nc.gpsimd.dma_start(w_in[:], gcol[:])
            nc.gpsimd.collective_compute(
                "AllReduce", mybir.AluOpType.add,
                replica_groups=[list(range(NCORES))],
                ins=[w_in.opt()], outs=[w_out.opt()],
            )

            xb16 = [[xbp.tile([C, CHUNK], bf16, name=f"xb_{b}_{k}", tag="xb")
                     for k in range(NCHUNK)] for b in range(B)]

            # ---- phase 1 + per-batch AllReduce ----
            ntile_c = CHUNK // C  # 16 n-tiles of 128 per chunk
            ntile = NCHUNK * ntile_c  # 144 per batch
            E_sb = []
            for b in range(B):
                e_ps = eps.tile([C, C], f32, name=f"e_ps{b}", tag="e")
                pend = []
                mm = 0

                def flush(e_ps=e_ps):
                    nonlocal mm
                    qt = pend.pop(0)
                    nc.tensor.matmul(e_ps[:], qt[:], qt[:],
                                     start=(mm == 0), stop=(mm == ntile - 1))
                    mm += 1

                for k in range(NCHUNK):
                    xt = xp.tile([C, CHUNK], f32, name=f"x_{b}_{k}", tag="x")
                    src = x_d[b, :, k * CHUNK:(k + 1) * CHUNK]
                    if b == 0 and k == 0:
                        # split the very first load so PE can start early
                        for s in range(2):
                            nc.sync.dma_start(
                                xt[:, s * 1024:(s + 1) * 1024],
                                x_d[0, :, s * 1024:(s + 1) * 1024])
                    else:
                        nc.sync.dma_start(xt[:], src)
                    for j in range(ntile_c):
                        t = k * ntile_c + j
                        tp = tps.tile([C, C], f32, name=f"tp_{b}_{t}",
                                      tag="tp")
                        nc.tensor.transpose(
                            tp[:], xt[:, j * C:(j + 1) * C], ident[:])
                        qt = qtp.tile([C, C], f32, name=f"qt_{b}_{t}",
                                      tag="qt")
                        nc.vector.tensor_copy(qt[:], tp[:])
                        pend.append(qt)
                        if len(pend) > PIPE:
                            flush()
                    # bf16 copy for phase 2 (ScalarE is idle in phase 1);
                    # after this the fp32 ring slot can be reused.
                    nc.scalar.copy(xb16[b][k][:], xt[:])
                while pend:
                    flush()
                e_cat = mp.tile([C, C], f32, name=f"e_cat{b}")
                nc.vector.tensor_copy(e_cat[:], e_ps[:])

                ar_in = dramp.tile([C, C], f32, name=f"ar_in{b}")
                ar_out = dramp.tile([C, C], f32, name=f"ar_out{b}",
                                    addr_space="Shared")
                # bounce DMAs on GPSIMD/SWDGE: the HWDGE (sync) ring is
                # strictly FIFO, so a collective-gated load there would
                # block all later chunk loads / output stores.
                nc.gpsimd.dma_start(ar_in[:], e_cat[:])
                nc.gpsimd.collective_compute(
                    "AllReduce", mybir.AluOpType.add,
                    replica_groups=[list(range(NCORES))],
                    ins=[ar_in.opt()], outs=[ar_out.opt()],
                )
                e_red = mp.tile([C, C], f32, name=f"e_red{b}")
                nc.gpsimd.dma_start(e_red[:], ar_out[:])
                E_sb.append(e_red)

            # ---- phase 2: softmax + apply, per batch ----
            def emit_softmax(b):
                E_b = E_sb[b][:]
                mcol = mp.tile([C, 1], f32, name=f"mcol{b}")
                nc.vector.tensor_reduce(mcol[:], E_b, axis=mybir.AxisListType.X,
                                        op=mybir.AluOpType.min)
                P_b = mp.tile([C, C], f32, name=f"P{b}")
                zcol = mp.tile([C, 1], f32, name=f"zcol{b}")
                # P = exp(min_row - E), zcol = rowsum(P); exponents <= 0.
                # P's diagonal is exp(min - ~+147000) == 0 exactly.
                nc.scalar.activation(P_b[:], E_b,
                                     mybir.ActivationFunctionType.Exp,
                                     bias=mcol[:], scale=-1.0,
                                     accum_out=zcol[:])
                rz = mp.tile([C, 1], f32, name=f"rz{b}")
                nc.vector.reciprocal(rz[:], zcol[:])
                scol = mp.tile([C, 1], f32, name=f"scol{b}")
                nc.vector.tensor_tensor(scol[:], rz[:], gcol[:],
                                        op=mybir.AluOpType.mult)
                # attn_s = (gamma/Z) * P + I  -> matmul computes x + gamma*attn@q
                nc.vector.tensor_scalar_mul(P_b[:], P_b[:], scol[:])
                nc.vector.tensor_add(P_b[:], P_b[:], ident[:])
                tp2 = tps.tile([C, C], f32, name=f"tpP{b}", tag="tp")
                nc.tensor.transpose(tp2[:], P_b[:], ident[:])
                attnT = mp.tile([C, C], bf16, name=f"attnT{b}")
                nc.vector.tensor_copy(attnT[:], tp2[:])  # fp32 psum -> bf16
                return attnT

            def emit_apply_chunk(b, attnT, k):
                ost = ostp.tile([C, CHUNK], f32, name=f"ost_{b}_{k}",
                                tag="ost")
                for j in range(CHUNK // OTILE):
                    op = ops.tile([C, OTILE], f32, name=f"op_{b}_{k}_{j}",
                                  tag="op")
                    nc.tensor.matmul(
                        op[:], attnT[:],
                        xb16[b][k][:, j * OTILE:(j + 1) * OTILE],
                        start=True, stop=True)
                    nc.vector.tensor_copy(
                        ost[:, j * OTILE:(j + 1) * OTILE], op[:])
                nc.sync.dma_start(o_d[b, :, k * CHUNK:(k + 1) * CHUNK],
                                  ost[:])

            attnT0 = emit_softmax(0)
            attnT1 = None
            for k in range(NCHUNK):
                emit_apply_chunk(0, attnT0, k)
                if k == 5:
                    # AR(b1) has completed by now; emitting softmax(b1) here
                    # keeps its DVE chain from queueing behind all of
                    # p2b0's evac copies.
                    attnT1 = emit_softmax(1)
            for k in range(NCHUNK):
                emit_apply_chunk(1, attnT1, k)

    _log("tile context done; bacc compile start")
    nc.compile()
    _log("bacc compile done")
    return nc


def _get_nc():
    if "nc" not in _compiled:
        _compiled["nc"] = _build()
    return _compiled["nc"]


def kernel(x, gamma, _trace=False, _tmpdir=None):
    from concourse import bass_utils

    x = np.ascontiguousarray(np.asarray(x), dtype=np.float32)
    gamma = np.asarray(gamma, dtype=np.float32)
    q = x.reshape(B, C, N)
    gcol = np.full((C, 1), gamma[0], dtype=np.float32)
    ident = np.eye(C, dtype=np.float32)

    in_maps = []
    for r in range(NCORES):
        in_maps.append({
            "x": np.ascontiguousarray(q[:, :, r * NLOC:(r + 1) * NLOC]),
            "gamma_col": gcol,
            "ident": ident,
        })

    nc = _get_nc()
    _log("launching run_bass_kernel_spmd")
    res = bass_utils.run_bass_kernel_spmd(
        nc, in_maps, core_ids=list(range(NCORES)), trace=_trace,
        tmpdir=_tmpdir)
    outs = [res.results[r]["out"] for r in range(NCORES)]
    full = np.concatenate(outs, axis=2).reshape(B, C, D, H, W)
    if _trace:
        return full.astype(np.float32, copy=False), res
    return full.astype(np.float32, copy=False)


# revision 25
# speedup vs baseline: 1.5324x; 1.0472x over previous
"""Channel-attention module kernel for 8 Trainium2 NeuronCores.

reference semantics (B=2, C=128, N=D*H*W=147456):
    q = x.reshape(B, C, N)
    energy = q @ q^T                  # [B, C, C]
    attn = softmax(rowmax(energy) - energy, axis=-1)
          = softmax(-energy, axis=-1)             (rowmax shift is a no-op)
    out = attn @ q
    return x + gamma * out

Sharding: sequence-parallel over N. Core r owns columns
[r*N/8, (r+1)*N/8) of q for both batches. Each core computes a partial
energy (contraction over its local n), a per-batch AllReduce sums the
tiny [C, C] energy across the 8 cores, each core then computes the
softmax redundantly and applies the attention to its local columns.

Pipelining: energy(b0) -> AR(b0) overlaps energy(b1); AR(b1) overlaps
phase2(b0).

Precision split:
  - energy contraction: true fp32 (softmax argmin gaps as small as 0.03
    on these inputs; one argmin flip alone is ~5% global rel err).
  - phase 2 (attn apply): bf16. The residual is folded into the
    attention matrix (attn_s = gamma/Z * P + I; P's diagonal is exactly
    0 because the energy diagonal ~ +N dominates), so phase 2 is
    out = attn_s @ q with q rounded to bf16 — error is linear, ~0.4%,
    far inside the 2e-2 gate. This makes phase-2 matmuls 4x faster than
    fp32 and lets the fp32 x chunks be freed after phase 1: x lives in
    a small fp32 ring; a resident bf16 copy (cast on the idle ScalarE
    during phase 1) feeds phase 2.
"""

import sys

sys.path.insert(0, "/opt/trn_rl_repo")

import numpy as np

B, C = 2, 128
D, H, W = 16, 96, 96
N = D * H * W  # 147456
NCORES = 8
NLOC = N // NCORES  # 18432
CHUNK = 2048
NCHUNK = NLOC // CHUNK  # 9
OTILE = 512
PIPE = 2  # transposes emitted ahead of their matmul (keeps PE fed)

_compiled = {}


def _log(msg):
    import time as _t
    print(f"[kernel {_t.strftime('%H:%M:%S')}] {msg}", flush=True)


def _build():
    import concourse.bacc as bacc
    import concourse.tile as tile
    import concourse.mybir as mybir

    _log("build start")

    f32 = mybir.dt.float32
    bf16 = mybir.dt.bfloat16
    nc = bacc.Bacc("TRN2", target_bir_lowering=False, debug=False,
                   num_devices=NCORES)

    x_d = nc.dram_tensor("x", [B, C, NLOC], f32, kind="ExternalInput").ap()
    g_d = nc.dram_tensor("gamma_col", [C, 1], f32, kind="ExternalInput").ap()
    id_d = nc.dram_tensor("ident", [C, C], f32, kind="ExternalInput").ap()
    o_d = nc.dram_tensor("out", [B, C, NLOC], f32, kind="ExternalOutput").ap()

    with tile.TileContext(nc) as tc:
        with (
            tc.tile_pool(name="xring", bufs=7) as xp,
            tc.tile_pool(name="xb16", bufs=B * NCHUNK) as xbp,
            tc.tile_pool(name="qt", bufs=6) as qtp,
            tc.tile_pool(name="tps", bufs=3, space="PSUM") as tps,
            tc.tile_pool(name="eps", bufs=2, space="PSUM") as eps,
            tc.tile_pool(name="ops", bufs=3, space="PSUM") as ops,
            tc.tile_pool(name="misc", bufs=1) as mp,
            tc.tile_pool(name="ost", bufs=3) as ostp,
            tc.tile_pool(name="dram", bufs=1, space="DRAM") as dramp,
        ):
            ident = mp.tile([C, C], f32, name="ident_sb")
            nc.sync.dma_start(ident[:], id_d[:])
            gcol = mp.tile([C, 1], f32, name="gcol")
            nc.sync.dma_start(gcol[:], g_d[:])

            # Warm-up collective: the FIRST collective on this runtime pays
            # a ~45us ncfw cold-start (hw-measured); later ones hit the
            # ~10us floor. Fire a tiny dummy AllReduce immediately so the
            # real per-batch AllReduces run warm.
            w_in = dramp.tile([C, 1], f32, name="w_in")
            w_out = dramp.tile([C, 1], f32, name="w_out", addr_space="Shared")
            nc.gpsimd.dma_start(w_in[:], gcol[:])
            nc.gpsimd.collective_compute(
                "AllReduce", mybir.AluOpType.add,
                replica_groups=[list(range(NCORES))],
                ins=[w_in.opt()], outs=[w_out.opt()],
            )

            xb16 = [[xbp.tile([C, CHUNK], bf16, name=f"xb_{b}_{k}", tag="xb")
                     for k in range(NCHUNK)] for b in range(B)]

            # ---- phase 1 + per-batch AllReduce ----
            ntile_c = CHUNK // C  # 16 n-tiles of 128 per chunk
            ntile = NCHUNK * ntile_c  # 144 per batch
            E_sb = []
            for b in range(B):
                e_ps = eps.tile([C, C], f32, name=f"e_ps{b}", tag="e")
                pend = []
                mm = 0

                def flush(e_ps=e_ps):
                    nonlocal mm
                    qt = pend.pop(0)
                    nc.tensor.matmul(e_ps[:], qt[:], qt[:],
                                     start=(mm == 0), stop=(mm == ntile - 1))
                    mm += 1

                for k in range(NCHUNK):
                    xt = xp.tile([C, CHUNK], f32, name=f"x_{b}_{k}", tag="x")
                    src = x_d[b, :, k * CHUNK:(k + 1) * CHUNK]
                    if b == 0 and k == 0:
                        # split the very first load so PE can start early
                        for s in range(2):
                            nc.sync.dma_start(
                                xt[:, s * 1024:(s + 1) * 1024],
                                x_d[0, :, s * 1024:(s + 1) * 1024])
                    else:
                        nc.sync.dma_start(xt[:], src)
                    for j in range(ntile_c):
                        t = k * ntile_c + j
                        tp = tps.tile([C, C], f32, name=f"tp_{b}_{t}",
                                      tag="tp")
                        nc.tensor.transpose(
                            tp[:], xt[:, j * C:(j + 1) * C], ident[:])
                        qt = qtp.tile([C, C], f32, name=f"qt_{b}_{t}",
                                      tag="qt")
                        nc.vector.tensor_copy(qt[:], tp[:])
                        pend.append(qt)
                        if len(pend) > PIPE:
                            flush()
                    # bf16 copy for phase 2 (ScalarE is idle in phase 1);
                    # after this the fp32 ring slot can be reused.
                    nc.scalar.copy(xb16[b][k][:], xt[:])
                while pend:
                    flush()
                e_cat = mp.tile([C, C], f32, name=f"e_cat{b}")
                nc.vector.tensor_copy(e_cat[:], e_ps[:])

                ar_in = dramp.tile([C, C], f32, name=f"ar_in{b}")
                ar_out = dramp.tile([C, C], f32, name=f"ar_out{b}",
                                    addr_space="Shared")
                # bounce DMAs on GPSIMD/SWDGE: the HWDGE (sync) ring is
                # strictly FIFO, so a collective-gated load there would
                # block all later chunk loads / output stores.
                nc.gpsimd.dma_start(ar_in[:], e_cat[:])
                nc.gpsimd.collective_compute(
                    "AllReduce", mybir.AluOpType.add,
                    replica_groups=[list(range(NCORES))],
                    ins=[ar_in.opt()], outs=[ar_out.opt()],
                )
                e_red = mp.tile([C, C], f32, name=f"e_red{b}")
                nc.gpsimd.dma_start(e_red[:], ar_out[:])
                E_sb.append(e_red)

            # ---- phase 2: softmax + apply, per batch ----
            def emit_softmax(b):
                E_b = E_sb[b][:]
                mcol = mp.tile([C, 1], f32, name=f"mcol{b}")
                nc.vector.tensor_reduce(mcol[:], E_b, axis=mybir.AxisListType.X,
                                        op=mybir.AluOpType.min)
                P_b = mp.tile([C, C], f32, name=f"P{b}")
                zcol = mp.tile([C, 1], f32, name=f"zcol{b}")
                # P = exp(min_row - E), zcol = rowsum(P); exponents <= 0.
                # P's diagonal is exp(min - ~+147000) == 0 exactly.
                nc.scalar.activation(P_b[:], E_b,
                                     mybir.ActivationFunctionType.Exp,
                                     bias=mcol[:], scale=-1.0,
                                     accum_out=zcol[:])
                rz = mp.tile([C, 1], f32, name=f"rz{b}")
                nc.vector.reciprocal(rz[:], zcol[:])
                scol = mp.tile([C, 1], f32, name=f"scol{b}")
                nc.vector.tensor_tensor(scol[:], rz[:], gcol[:],
                                        op=mybir.AluOpType.mult)
                # attn_s = (gamma/Z) * P + I  -> matmul computes x + gamma*attn@q
                nc.vector.tensor_scalar_mul(P_b[:], P_b[:], scol[:])
                nc.vector.tensor_add(P_b[:], P_b[:], ident[:])
                tp2 = tps.tile([C, C], f32, name=f"tpP{b}", tag="tp")
                nc.tensor.transpose(tp2[:], P_b[:], ident[:])
                attnT = mp.tile([C, C], bf16, name=f"attnT{b}")
                nc.vector.tensor_copy(attnT[:], tp2[:])  # fp32 psum -> bf16
                return attnT

            def emit_apply_chunk(b, attnT, k):
                ost = ostp.tile([C, CHUNK], f32, name=f"ost_{b}_{k}",
                                tag="ost")
                for j in range(CHUNK // OTILE):
                    op = ops.tile([C, OTILE], f32, name=f"op_{b}_{k}_{j}",
                                  tag="op")
                    nc.tensor.matmul(
                        op[:], attnT[:],
                        xb16[b][k][:, j * OTILE:(j + 1) * OTILE],
                        start=True, stop=True)
                    nc.vector.tensor_copy(
                        ost[:, j * OTILE:(j + 1) * OTILE], op[:])
                nc.sync.dma_start(o_d[b, :, k * CHUNK:(k + 1) * CHUNK],
                                  ost[:])

            for b in range(B):
                attnT = emit_softmax(b)
                for k in range(NCHUNK):
                    emit_apply_chunk(b, attnT, k)

    _log("tile context done; bacc compile start")
    nc.compile()
    _log("bacc compile done")
    return nc


def _get_nc():
    if "nc" not in _compiled:
        _compiled["nc"] = _build()
    return _compiled["nc"]


def kernel(x, gamma, _trace=False, _tmpdir=None):
    from concourse import bass_utils

    x = np.ascontiguousarray(np.asarray(x), dtype=np.float32)
    gamma = np.asarray(gamma, dtype=np.float32)
    q = x.reshape(B, C, N)
    gcol = np.full((C, 1), gamma[0], dtype=np.float32)
    ident = np.eye(C, dtype=np.float32)

    in_maps = []
    for r in range(NCORES):
        in_maps.append({
            "x": np.ascontiguousarray(q[:, :, r * NLOC:(r + 1) * NLOC]),
            "gamma_col": gcol,
            "ident": ident,
        })

    nc = _get_nc()
    _log("launching run_bass_kernel_spmd")
    res = bass_utils.run_bass_kernel_spmd(
        nc, in_maps, core_ids=list(range(NCORES)), trace=_trace,
        tmpdir=_tmpdir)
    outs = [res.results[r]["out"] for r in range(NCORES)]
    full = np.concatenate(outs, axis=2).reshape(B, C, D, H, W)
    if _trace:
        return full.astype(np.float32, copy=False), res
    return full.astype(np.float32, copy=False)


# revision 26
# speedup vs baseline: 1.5570x; 1.0161x over previous
"""Channel-attention module kernel for 8 Trainium2 NeuronCores.

reference semantics (B=2, C=128, N=D*H*W=147456):
    q = x.reshape(B, C, N)
    energy = q @ q^T                  # [B, C, C]
    attn = softmax(rowmax(energy) - energy, axis=-1)
          = softmax(-energy, axis=-1)             (rowmax shift is a no-op)
    out = attn @ q
    return x + gamma * out

Sharding: sequence-parallel over N. Core r owns columns
[r*N/8, (r+1)*N/8) of q for both batches. Each core computes a partial
energy (contraction over its local n), a per-batch AllReduce sums the
tiny [C, C] energy across the 8 cores, each core then computes the
softmax redundantly and applies the attention to its local columns.

Pipelining: energy(b0) -> AR(b0) overlaps energy(b1); AR(b1) overlaps
phase2(b0).

Precision split:
  - energy contraction: true fp32 (softmax argmin gaps as small as 0.03
    on these inputs; one argmin flip alone is ~5% global rel err).
  - phase 2 (attn apply): bf16. The residual is folded into the
    attention matrix (attn_s = gamma/Z * P + I; P's diagonal is exactly
    0 because the energy diagonal ~ +N dominates), so phase 2 is
    out = attn_s @ q with q rounded to bf16 — error is linear, ~0.4%,
    far inside the 2e-2 gate. This makes phase-2 matmuls 4x faster than
    fp32 and lets the fp32 x chunks be freed after phase 1: x lives in
    a small fp32 ring; a resident bf16 copy (cast on the idle ScalarE
    during phase 1) feeds phase 2.
"""

import sys

sys.path.insert(0, "/opt/trn_rl_repo")

import numpy as np

B, C = 2, 128
D, H, W = 16, 96, 96
N = D * H * W  # 147456
NCORES = 8
NLOC = N // NCORES  # 18432
CHUNK = 2048
NCHUNK = NLOC // CHUNK  # 9
OTILE = 512
PIPE = 2  # transposes emitted ahead of their matmul (keeps PE fed)

_compiled = {}


def _log(msg):
    import time as _t
    print(f"[kernel {_t.strftime('%H:%M:%S')}] {msg}", flush=True)


def _build():
    import concourse.bacc as bacc
    import concourse.tile as tile
    import concourse.mybir as mybir

    _log("build start")

    f32 = mybir.dt.float32
    f16 = mybir.dt.float16
    bf16 = mybir.dt.bfloat16
    nc = bacc.Bacc("TRN2", target_bir_lowering=False, debug=False,
                   num_devices=NCORES)

    x_d = nc.dram_tensor("x", [B, C, NLOC], f32, kind="ExternalInput").ap()
    g_d = nc.dram_tensor("gamma_col", [C, 1], f32, kind="ExternalInput").ap()
    id_d = nc.dram_tensor("ident", [C, C], f32, kind="ExternalInput").ap()
    o_d = nc.dram_tensor("out", [B, C, NLOC], f16, kind="ExternalOutput").ap()

    with tile.TileContext(nc) as tc:
        with (
            tc.tile_pool(name="xring", bufs=8) as xp,
            tc.tile_pool(name="xb16", bufs=B * NCHUNK) as xbp,
            tc.tile_pool(name="qt", bufs=6) as qtp,
            tc.tile_pool(name="tps", bufs=3, space="PSUM") as tps,
            tc.tile_pool(name="eps", bufs=2, space="PSUM") as eps,
            tc.tile_pool(name="ops", bufs=3, space="PSUM") as ops,
            tc.tile_pool(name="misc", bufs=1) as mp,
            tc.tile_pool(name="ost", bufs=3) as ostp,
            tc.tile_pool(name="dram", bufs=1, space="DRAM") as dramp,
        ):
            ident = mp.tile([C, C], f32, name="ident_sb")
            nc.sync.dma_start(ident[:], id_d[:])
            gcol = mp.tile([C, 1], f32, name="gcol")
            nc.sync.dma_start(gcol[:], g_d[:])

            # Warm-up collective: the FIRST collective on this runtime pays
            # a ~45us ncfw cold-start (hw-measured); later ones hit the
            # ~10us floor. Fire a tiny dummy AllReduce immediately so the
            # real per-batch AllReduces run warm.
            w_in = dramp.tile([C, 1], f32, name="w_in")
            w_out = dramp.tile([C, 1], f32, name="w_out", addr_space="Shared")
            nc.gpsimd.dma_start(w_in[:], gcol[:])
            nc.gpsimd.collective_compute(
                "AllReduce", mybir.AluOpType.add,
                replica_groups=[list(range(NCORES))],
                ins=[w_in.opt()], outs=[w_out.opt()],
            )

            xb16 = [[xbp.tile([C, CHUNK], bf16, name=f"xb_{b}_{k}", tag="xb")
                     for k in range(NCHUNK)] for b in range(B)]

            # ---- phase 1 + per-batch AllReduce ----
            ntile_c = CHUNK // C  # 16 n-tiles of 128 per chunk
            ntile = NCHUNK * ntile_c  # 144 per batch
            E_sb = []
            for b in range(B):
                e_ps = eps.tile([C, C], f32, name=f"e_ps{b}", tag="e")
                pend = []
                mm = 0

                def flush(e_ps=e_ps):
                    nonlocal mm
                    qt = pend.pop(0)
                    nc.tensor.matmul(e_ps[:], qt[:], qt[:],
                                     start=(mm == 0), stop=(mm == ntile - 1))
                    mm += 1

                for k in range(NCHUNK):
                    xt = xp.tile([C, CHUNK], f32, name=f"x_{b}_{k}", tag="x")
                    src = x_d[b, :, k * CHUNK:(k + 1) * CHUNK]
                    if b == 0 and k == 0:
                        # split the very first load so PE can start early
                        for s in range(2):
                            nc.sync.dma_start(
                                xt[:, s * 1024:(s + 1) * 1024],
                                x_d[0, :, s * 1024:(s + 1) * 1024])
                    else:
                        nc.sync.dma_start(xt[:], src)
                    for j in range(ntile_c):
                        t = k * ntile_c + j
                        tp = tps.tile([C, C], f32, name=f"tp_{b}_{t}",
                                      tag="tp")
                        nc.tensor.transpose(
                            tp[:], xt[:, j * C:(j + 1) * C], ident[:])
                        qt = qtp.tile([C, C], f32, name=f"qt_{b}_{t}",
                                      tag="qt")
                        nc.vector.tensor_copy(qt[:], tp[:])
                        pend.append(qt)
                        if len(pend) > PIPE:
                            flush()
                    # bf16 copy for phase 2 (ScalarE is idle in phase 1);
                    # after this the fp32 ring slot can be reused.
                    nc.scalar.copy(xb16[b][k][:], xt[:])
                while pend:
                    flush()
                e_cat = mp.tile([C, C], f32, name=f"e_cat{b}")
                nc.vector.tensor_copy(e_cat[:], e_ps[:])

                ar_in = dramp.tile([C, C], f32, name=f"ar_in{b}")
                ar_out = dramp.tile([C, C], f32, name=f"ar_out{b}",
                                    addr_space="Shared")
                # bounce DMAs on GPSIMD/SWDGE: the HWDGE (sync) ring is
                # strictly FIFO, so a collective-gated load there would
                # block all later chunk loads / output stores.
                nc.gpsimd.dma_start(ar_in[:], e_cat[:])
                nc.gpsimd.collective_compute(
                    "AllReduce", mybir.AluOpType.add,
                    replica_groups=[list(range(NCORES))],
                    ins=[ar_in.opt()], outs=[ar_out.opt()],
                )
                e_red = mp.tile([C, C], f32, name=f"e_red{b}")
                nc.gpsimd.dma_start(e_red[:], ar_out[:])
                E_sb.append(e_red)

            # ---- phase 2: softmax + apply, per batch ----
            def emit_softmax(b):
                E_b = E_sb[b][:]
                mcol = mp.tile([C, 1], f32, name=f"mcol{b}")
                nc.vector.tensor_reduce(mcol[:], E_b, axis=mybir.AxisListType.X,
                                        op=mybir.AluOpType.min)
                P_b = mp.tile([C, C], f32, name=f"P{b}")
                zcol = mp.tile([C, 1], f32, name=f"zcol{b}")
                # P = exp(min_row - E), zcol = rowsum(P); exponents <= 0.
                # P's diagonal is exp(min - ~+147000) == 0 exactly.
                nc.scalar.activation(P_b[:], E_b,
                                     mybir.ActivationFunctionType.Exp,
                                     bias=mcol[:], scale=-1.0,
                                     accum_out=zcol[:])
                rz = mp.tile([C, 1], f32, name=f"rz{b}")
                nc.vector.reciprocal(rz[:], zcol[:])
                scol = mp.tile([C, 1], f32, name=f"scol{b}")
                nc.vector.tensor_tensor(scol[:], rz[:], gcol[:],
                                        op=mybir.AluOpType.mult)
                # attn_s = (gamma/Z) * P + I  -> matmul computes x + gamma*attn@q
                nc.vector.tensor_scalar_mul(P_b[:], P_b[:], scol[:])
                nc.vector.tensor_add(P_b[:], P_b[:], ident[:])
                tp2 = tps.tile([C, C], f32, name=f"tpP{b}", tag="tp")
                nc.tensor.transpose(tp2[:], P_b[:], ident[:])
                attnT = mp.tile([C, C], bf16, name=f"attnT{b}")
                nc.vector.tensor_copy(attnT[:], tp2[:])  # fp32 psum -> bf16
                return attnT

            def emit_apply_chunk(b, attnT, k):
                ost = ostp.tile([C, CHUNK], f16, name=f"ost_{b}_{k}",
                                tag="ost")
                for j in range(CHUNK // OTILE):
                    op = ops.tile([C, OTILE], f32, name=f"op_{b}_{k}_{j}",
                                  tag="op")
                    nc.tensor.matmul(
                        op[:], attnT[:],
                        xb16[b][k][:, j * OTILE:(j + 1) * OTILE],
                        start=True, stop=True)
                    dst = ost[:, j * OTILE:(j + 1) * OTILE]
                    if j % 2 == 0:
                        nc.vector.tensor_copy(dst, op[:])
                    else:
                        nc.scalar.copy(dst, op[:])
                nc.sync.dma_start(o_d[b, :, k * CHUNK:(k + 1) * CHUNK],
                                  ost[:])

            for b in range(B):
                attnT = emit_softmax(b)
                for k in range(NCHUNK):
                    emit_apply_chunk(b, attnT, k)

    _log("tile context done; bacc compile start")
    nc.compile()
    _log("bacc compile done")
    return nc


def _get_nc():
    if "nc" not in _compiled:
        _compiled["nc"] = _build()
    return _compiled["nc"]


def kernel(x, gamma, _trace=False, _tmpdir=None):
    from concourse import bass_utils

    x = np.ascontiguousarray(np.asarray(x), dtype=np.float32)
    gamma = np.asarray(gamma, dtype=np.float32)
    q = x.reshape(B, C, N)
    gcol = np.full((C, 1), gamma[0], dtype=np.float32)
    ident = np.eye(C, dtype=np.float32)

    in_maps = []
    for r in range(NCORES):
        in_maps.append({
            "x": np.ascontiguousarray(q[:, :, r * NLOC:(r + 1) * NLOC]),
            "gamma_col": gcol,
            "ident": ident,
        })

    nc = _get_nc()
    _log("launching run_bass_kernel_spmd")
    res = bass_utils.run_bass_kernel_spmd(
        nc, in_maps, core_ids=list(range(NCORES)), trace=_trace,
        tmpdir=_tmpdir)
    outs = [res.results[r]["out"] for r in range(NCORES)]
    full = np.concatenate(outs, axis=2).astype(np.float32)
    full = full.reshape(B, C, D, H, W)
    if _trace:
        return full.astype(np.float32, copy=False), res
    return full.astype(np.float32, copy=False)
